# revision 15
# baseline (speedup 1.0000x reference)
"""Color-preserving non-local block (dense softmax attention, N=9216, I=32)
distributed over 8 TRN2 NeuronCores.

Sharding: data-parallel over batch B=2 (4 cores per batch) x sequence-parallel
over the N=9216 query rows (2304 rows per core).  Each core receives the full
[C, N] image of its batch (rolled so its query slice starts at column 0 --
softmax over keys is permutation-invariant, so rolling the key axis is free),
computes the projections redundantly, and produces its [C, 2304] output slice.
No collectives are needed.

v4: three-engine pipeline.
  QK: 3-way row-tiled K=32 matmuls (strips at partitions 0/32/64 run
      concurrently in the PE array) -> ~3x less PE time than the v3
      K=128-replicated scheme.  Scores come out at 1x scale.
  exp: split between ACT (table exp -> fp8e4 directly) and DVE (Schraudolph
      bit-trick: code = st*8/(T ln2) + 56 written as int8 and reinterpreted
      as fp8e4).  Softmax normalization cancels the Schraudolph bias almost
      exactly; measured end-to-end rms vs the f32 reference is ~5e-6.
  PV: fp8 DoubleRow matmuls contract TWO 128-deep kv tiles per instruction
      (g16 = 16*g in fp8e4 with a ones*16 column for the denominator; the
      16x cancels in the normalize, guards fp8 subnormals).
  1/denominator: d/d0 deviates <6% from 1, so 1/d = (q+0.75)/d0 with
      q = (d/d0 - 1.5)^2 (one ACT Square op) to 3rd-order accuracy; the
      1/d0 folds into the gate scalar.
  Epilogue element-wise work runs on GpSimd; the gate's global pooling is
      GpSimd partial reduces over the bf16 x copies as their DMAs land.
"""

import sys

for _p in ("/opt/trn_rl_repo",):
    if _p not in sys.path:
        sys.path.insert(0, _p)

import math

import numpy as np
import ml_dtypes

import concourse.bass as bass
import concourse.tile as tile
from concourse import bacc, mybir
from concourse.bass import ts, ds
from concourse.bass_utils import run_bass_kernel_spmd

F32 = mybir.dt.float32
BF16 = mybir.dt.bfloat16
F8 = mybir.dt.float8e4
I8 = mybir.dt.int8

B, C, H, W = 2, 64, 96, 96
N = H * W                    # 9216
I = 32                       # inter dim
NB = 16                      # gate bottleneck dim
NCORES = 8
CPB = NCORES // B            # cores per batch = 4
QPC = N // CPB               # 2304 query rows per core
KT = 128                     # kv tile
NKV = N // KT                # 72
NPAIR = NKV // 2             # 36 DoubleRow pairs
SLOT = 3                     # kv tiles per QK slot (3-way row tiling)
NSLOT = NKV // SLOT          # 24 slots
NW = NSLOT // 2              # 12 windows (2 slots = 3 PV pairs each)
QCH = 512                    # q chunk (PSUM free dim)
GTS = 48                     # gt2 plane stride (34 used; %16==0 for DoubleRow)
TEMP = 1.5
PR = 0.8
D0 = 16.0 * N                # denominator scale (ones column is 16.0)
A_SCH = 8.0 / (TEMP * math.log(2.0))   # Schraudolph slope for fp8e4 codes
B_SCH = 56.0                           # 7 (bias) * 8

# exp-slot engine assignment: even slots (PSUM buffer A) on ACT, odd slots
# (buffer B) on DVE.  Each window's two exps then always run in parallel on
# different engines, and the pipeline's critical cycle (QK slot -> exp ->
# QK slot two later, bounded by the 2 st buffers) takes the cheaper engine
# on each buffer lane.


def _chunks():
    out = []
    q = 0
    while q < QPC:
        out.append((q, min(QCH, QPC - q)))
        q += QCH
    return out


def _emit(tc, nc, dr, out_d):
    mm = nc.tensor.matmul
    with (
        tc.tile_pool(name="consts", bufs=1) as consts,
        tc.tile_pool(name="work", bufs=2) as work,
        tc.tile_pool(name="epool", bufs=3) as epool,
    ):
        # ---- persistent SBUF tensors -------------------------------------
        xb_sb = consts.tile([C, N], F32)        # residual path
        xbh2_sb = consts.tile([128, N], BF16)   # x stacked twice on partitions
        wbf_sb = consts.tile([128, 352], BF16)  # bf16 weight blob
        thw_sb = wbf_sb[:, 0:128]               # 0.5 * theta_w^T tiled (2, 4)
        phw_sb = wbf_sb[:, 128:256]             # 0.5 * phi_w^T tiled (2, 4)
        gw_sb = wbf_sb[:, 256:288]              # 0.5 * g_w^T tiled (2, 1)
        ww_sb = wbf_sb[:I, 288:352]             # W_w^T
        wf32_sb = consts.tile([C, 82], F32)     # f32 weight blob
        c1w_sb = wf32_sb[:, 0:NB]
        c1b_sb = wf32_sb[:NB, NB : NB + 1]
        c2w_sb = wf32_sb[:NB, 17:81]
        nc2b_sb = wf32_sb[:, 81:82]

        theta4_sb = consts.tile([128, QPC], BF16)   # theta replicated x4
        phi4_sb = consts.tile([128, N], BF16)       # phi replicated x4
        gt2_sb = consts.tile([128, NPAIR, 2, GTS], F8)  # g*16 | ones*16 | 0pad
        gate_sb = consts.tile([C, 1], F32)
        pool4_sb = consts.tile([C, 4], F32)
        pool_sb = consts.tile([C, 1], F32)
        h_sb = consts.tile([NB, 1], F32)
        eg_sb = consts.tile([C, 1], F32)
        nbias_sb = consts.tile([1, 1], F32)     # -1.5 bias for the Square op
        nc.vector.memset(nbias_sb, -1.5)

        nc.sync.dma_start(out=wbf_sb, in_=dr["wbf"])
        for k in range(CPB):
            s0 = k * QPC
            nc.sync.dma_start(out=xbh2_sb[:, s0 : s0 + QPC],
                              in_=dr["xbh2"][:, s0 : s0 + QPC])
        nc.sync.dma_start(out=xb_sb, in_=dr["xb"])
        nc.sync.dma_start(out=wf32_sb, in_=dr["wf32"])

        # ones*16 column + zero pad of the DoubleRow g blob (GpSimd memsets:
        # keep the DVE free for the projection casts that gate the main loop)
        nc.gpsimd.memset(gt2_sb[:, :, :, I : I + 1], 16.0)
        nc.gpsimd.memset(gt2_sb[:, :, :, I + 1 : GTS], 0.0)

        # ---- prologue projections (all K=128) ----------------------------
        with tc.tile_pool(name="ppsum", bufs=3, space="PSUM") as pp:
            def proj(dst, w_sb, total, eng_split):
                # two 512-wide matmuls into one 2-bank tile, one wide cast;
                # psum->sbuf casts alternate ACT/DVE to halve the copy tail
                q = 0
                it = 0
                while q < total:
                    pt = pp.tile([128, 2, QCH], F32, tag="pp")
                    n = 0
                    for j in range(2):
                        if q + n < total:
                            w = min(QCH, total - (q + n))
                            mm(out=pt[:, j, :w], lhsT=w_sb,
                               rhs=xbh2_sb[:, ds(q + n, w)],
                               start=True, stop=True)
                            n += w
                    src = pt.rearrange("p a b -> p (a b)")[:, :n]
                    if it % 2 == eng_split:
                        nc.scalar.copy(out=dst[:, ds(q, n)], in_=src)
                    else:
                        nc.vector.tensor_copy(out=dst[:, ds(q, n)], in_=src)
                    q += n
                    it += 1
            proj(theta4_sb, thw_sb, QPC, 0)
            proj(phi4_sb, phw_sb, N, 1)
            done = 0
            while done < NKV:
                nt = min(16, NKV - done)
                pt = pp.tile([128, QCH], F32, tag="pp")
                for k in range(nt):
                    t = done + k
                    mm(out=pt[:, ts(k, I)], lhsT=xbh2_sb[:, ts(t, KT)],
                       rhs=gw_sb, start=True, stop=True)
                nc.vector.tensor_scalar_mul(
                    gt2_sb[:, done // 2 : (done + nt) // 2, :, 0:I],
                    pt[:, : nt * I].rearrange("p (r j i) -> p r j i", j=2, i=I),
                    16.0,
                )
                done += nt
            # gate pooling: bf16 partial reduces (2x DVE rate), after the
            # projection copies so they don't delay the main loop's start
            for k in range(CPB):
                nc.vector.reduce_sum(out=pool4_sb[:, k : k + 1],
                                     in_=xbh2_sb[:C, ts(k, QPC)],
                                     axis=mybir.AxisListType.X)
            nc.vector.reduce_sum(out=pool_sb, in_=pool4_sb,
                                 axis=mybir.AxisListType.X)

        # ---- main loop ---------------------------------------------------
        with (
            tc.tile_pool(name="pst", bufs=2, space="PSUM") as pst,
            tc.tile_pool(name="py", bufs=1, space="PSUM") as py,
            tc.tile_pool(name="pmisc", bufs=1, space="PSUM") as pmisc,
        ):
            def emit_gate():
                # channel gate; emitted after chunk 0's first windows so its
                # matmuls never block the PE queue ahead of the main stream.
                # gate_sb = PR * sigmoid(z) / D0  (the 1/D0 of the softmax
                # denominator normalize folds in here).
                h_ps = pmisc.tile([128, QCH], F32, tag="m")
                mm(out=h_ps[:NB, 0:1], lhsT=c1w_sb, rhs=pool_sb,
                   start=True, stop=True)
                nc.scalar.activation(out=h_sb, in_=h_ps[:NB, 0:1],
                                     func=mybir.ActivationFunctionType.Relu,
                                     bias=c1b_sb, scale=1.0 / float(N))
                z_ps = pmisc.tile([128, QCH], F32, tag="m")
                mm(out=z_ps[:C, 0:1], lhsT=c2w_sb, rhs=h_sb,
                   start=True, stop=True)
                nc.scalar.activation(out=eg_sb, in_=z_ps[:C, 0:1],
                                     func=mybir.ActivationFunctionType.Exp,
                                     bias=nc2b_sb, scale=-1.0)
                nc.vector.tensor_scalar_add(gate_sb, eg_sb, 1.0)
                nc.vector.reciprocal(out=gate_sb, in_=gate_sb)
                nc.vector.tensor_scalar_mul(gate_sb, gate_sb, PR / D0)

            def _epi_head(y_ps, qn):
                ysum = work.tile([I, QCH], BF16, tag="ysum")
                nc.vector.tensor_copy(out=ysum[:, :qn], in_=y_ps[:I, :qn])
                # q = (d/D0 - 1.5)^2, read straight from the PSUM denom row;
                # later gate*(q + 0.75)*o = gate_full*o/d to 3rd order
                q_sb = work.tile([1, QCH], F32, tag="q")
                nc.scalar.activation(out=q_sb[:, :qn],
                                     in_=y_ps[I : I + 1, :qn],
                                     func=mybir.ActivationFunctionType.Square,
                                     bias=nbias_sb, scale=1.0 / D0)
                bc = work.tile([C, QCH], F32, tag="bc")
                nc.gpsimd.partition_broadcast(bc[:, :qn], q_sb[:, :qn])
                return ysum, bc

            def _epi_tail(qs, qn, ysum, bc):
                o_ps = pmisc.tile([128, QCH], F32, tag="m")
                mm(out=o_ps[:C, :qn], lhsT=ww_sb, rhs=ysum[:, :qn],
                   start=True, stop=True)
                t1 = work.tile([C, QCH], F32, tag="t1")
                nc.vector.scalar_tensor_tensor(
                    out=t1[:, :qn], in0=bc[:, :qn], scalar=0.75,
                    in1=o_ps[:C, :qn],
                    op0=mybir.AluOpType.add, op1=mybir.AluOpType.mult)
                out_sb = work.tile([C, QCH], F32, tag="out")
                nc.vector.scalar_tensor_tensor(
                    out=out_sb[:, :qn], in0=t1[:, :qn], scalar=gate_sb,
                    in1=xb_sb[:, ds(qs, qn)],
                    op0=mybir.AluOpType.mult, op1=mybir.AluOpType.add)
                nc.sync.dma_start(out=out_d[:, ds(qs, qn)],
                                  in_=out_sb[:, :qn])

            chunks = _chunks()
            pend_pv = None     # last window's PV matmuls (deferred 1 window)
            pend_head = None   # previous chunk's epilogue head
            pend_tail = None   # previous chunk's epilogue tail
            for ci, (qs, qn) in enumerate(chunks):
                y_ps = py.tile([I + 2, QCH], F32, tag="y")
                # previous chunk's last PV + epilogue head go FIRST so the
                # head's copies land at the front of the DVE/ACT queues and
                # release the y bank before this chunk's PV pair 0 needs it.
                if pend_head is not None:
                    pend_head()
                    pend_head = None
                for w in range(NW):
                    e_t = epool.tile([128, 2 * SLOT, QCH], F8, tag="e")
                    for s2 in range(2):
                        slot = 2 * w + s2
                        st = pst.tile([128, SLOT, QCH], F32, tag="st")
                        for j in range(SLOT):
                            t = SLOT * slot + j
                            mm(out=st[:, j, :qn],
                               lhsT=phi4_sb[32 * j : 32 * j + 32, ts(t, KT)],
                               rhs=theta4_sb[32 * j : 32 * j + 32, ds(qs, qn)],
                               start=True, stop=True)
                        if qn == QCH:
                            # flat 2D APs (contiguous planes) stream best
                            eo = e_t[:, SLOT * s2 : SLOT * s2 + SLOT, :]
                            eo = eo.rearrange("p a b -> p (a b)")
                            si = st.rearrange("p a b -> p (a b)")
                        else:
                            eo = e_t[:, SLOT * s2 : SLOT * s2 + SLOT, :qn]
                            si = st[:, :, :qn]
                        if slot % 2 == 0:
                            nc.scalar.activation(
                                out=eo, in_=si,
                                func=mybir.ActivationFunctionType.Exp,
                                scale=1.0 / TEMP)
                        else:
                            nc.vector.tensor_scalar(
                                out=eo.bitcast(I8), in0=si,
                                scalar1=A_SCH, scalar2=B_SCH,
                                op0=mybir.AluOpType.mult,
                                op1=mybir.AluOpType.add)
                        # previous window's PV pairs go between this window's
                        # two QK slots: fills the PE's wait on the st buffer
                        if s2 == 0 and pend_pv is not None:
                            pend_pv()
                            pend_pv = None
                    if pend_tail is not None and w == 3:
                        pend_tail()
                        pend_tail = None

                    def _pv(e_t=e_t, w=w, y_ps=y_ps, qn=qn):
                        for p in range(3):
                            pr = 3 * w + p
                            mm(out=y_ps[: I + 2, :qn],
                               lhsT=gt2_sb[:, pr, :, : I + 2],
                               rhs=e_t[:, 2 * p : 2 * p + 2, :qn],
                               start=(pr == 0), stop=(pr == NPAIR - 1),
                               perf_mode=mybir.MatmulPerfMode.DoubleRow)
                    pend_pv = _pv
                if ci == 0:
                    emit_gate()

                if ci + 1 < len(chunks):
                    # defer: last window's PV + head into next chunk's stream
                    def _head(y_ps=y_ps, qs=qs, qn=qn, pv=pend_pv):
                        pv()
                        ysum, bc = _epi_head(y_ps, qn)

                        def _tail(qs=qs, qn=qn, ysum=ysum, bc=bc):
                            _epi_tail(qs, qn, ysum, bc)

                        nonlocal pend_tail
                        pend_tail = _tail
                    pend_pv = None
                    pend_head = _head
                else:
                    pend_pv()
                    pend_pv = None
                    ysum, bc = _epi_head(y_ps, qn)
                    _epi_tail(qs, qn, ysum, bc)
            if pend_head is not None:
                pend_head()
            if pend_tail is not None:
                pend_tail()


def build():
    nc = bacc.Bacc("TRN2", target_bir_lowering=False, debug=False)
    names = {
        "xb": ([C, N], F32), "xbh2": ([128, N], BF16),
        "wbf": ([128, 352], BF16), "wf32": ([C, 82], F32),
    }
    dr = {k: nc.dram_tensor(k, shp, dt, kind="ExternalInput").ap()
          for k, (shp, dt) in names.items()}
    out_d = nc.dram_tensor("out", [C, QPC], F32, kind="ExternalOutput").ap()
    with tile.TileContext(nc) as tc:
        _emit(tc, nc, dr, out_d)
    nc.compile()
    return nc


_NC = None


def _get_nc():
    global _NC
    if _NC is None:
        _NC = build()
    return _NC


def make_in_maps(inputs):
    bf = ml_dtypes.bfloat16
    xf = np.ascontiguousarray(np.asarray(inputs["x"], np.float32).reshape(B, C, N))
    thwT = np.asarray(inputs["theta_w"], np.float32).T        # [C, I]
    phwT = np.asarray(inputs["phi_w"], np.float32).T
    gwT = np.asarray(inputs["g_w"], np.float32).T
    wbf = np.zeros((128, 352), np.float32)
    wbf[:, 0:128] = np.tile(thwT, (2, 4)) * 0.5
    wbf[:, 128:256] = np.tile(phwT, (2, 4)) * 0.5
    wbf[:, 256:288] = np.tile(gwT, (2, 1)) * 0.5
    wbf[:I, 288:352] = np.asarray(inputs["W_w"], np.float32).T
    wf32 = np.zeros((C, 82), np.float32)
    wf32[:, 0:NB] = np.asarray(inputs["cg1_w"], np.float32).T
    wf32[:NB, NB] = np.asarray(inputs["cg1_b"], np.float32)
    wf32[:NB, 17:81] = np.asarray(inputs["cg2_w"], np.float32).T
    wf32[:, 81] = -np.asarray(inputs["cg2_b"], np.float32)
    shared = {"wbf": wbf.astype(bf), "wf32": wf32}
    in_maps = []
    for core in range(NCORES):
        b, q0 = core // CPB, (core % CPB) * QPC
        m = dict(shared)
        xr = np.ascontiguousarray(np.roll(xf[b], -q0, axis=1))
        m["xb"] = xr
        m["xbh2"] = np.ascontiguousarray(np.tile(xr, (2, 1))).astype(bf)
        in_maps.append(m)
    return in_maps


def gather(results):
    y = np.empty((B, C, N), np.float32)
    for core in range(NCORES):
        b, q0 = core // CPB, (core % CPB) * QPC
        y[b][:, q0 : q0 + QPC] = results[core]["out"]
    return y.reshape(B, C, H, W)


def run(inputs, trace=False, **kw):
    res = run_bass_kernel_spmd(_get_nc(), make_in_maps(inputs),
                               core_ids=list(range(NCORES)), trace=trace, **kw)
    return gather(res.results), res


def kernel(**inputs):
    out, _ = run(inputs)
    return out


# revision 16
# speedup vs baseline: 1.2552x; 1.2552x over previous
"""Color-preserving non-local block (dense softmax attention, N=9216, I=32)
distributed over 8 TRN2 NeuronCores.

Sharding: data-parallel over batch B=2 (4 cores per batch) x sequence-parallel
over the N=9216 query rows (2304 rows per core).  Each core receives the full
[C, N] image of its batch (rolled so its query slice starts at column 0 --
softmax over keys is permutation-invariant, so rolling the key axis is free),
computes the projections redundantly, and produces its [C, 2304] output slice.
No collectives are needed.

v4: three-engine pipeline.
  QK: 3-way row-tiled K=32 matmuls (strips at partitions 0/32/64 run
      concurrently in the PE array) -> ~3x less PE time than the v3
      K=128-replicated scheme.  Scores come out at 1x scale.
  exp: split between ACT (table exp -> fp8e4 directly) and DVE (Schraudolph
      bit-trick: code = st*8/(T ln2) + 56 written as int8 and reinterpreted
      as fp8e4).  Softmax normalization cancels the Schraudolph bias almost
      exactly; measured end-to-end rms vs the f32 reference is ~5e-6.
  PV: fp8 DoubleRow matmuls contract TWO 128-deep kv tiles per instruction
      (g16 = 16*g in fp8e4 with a ones*16 column for the denominator; the
      16x cancels in the normalize, guards fp8 subnormals).
  1/denominator: d/d0 deviates <6% from 1, so 1/d = (q+0.75)/d0 with
      q = (d/d0 - 1.5)^2 (one ACT Square op) to 3rd-order accuracy; the
      1/d0 folds into the gate scalar.
  Epilogue element-wise work runs on GpSimd; the gate's global pooling is
      GpSimd partial reduces over the bf16 x copies as their DMAs land.
"""

import sys

for _p in ("/opt/trn_rl_repo",):
    if _p not in sys.path:
        sys.path.insert(0, _p)

import math

import numpy as np
import ml_dtypes

import concourse.bass as bass
import concourse.tile as tile
from concourse import bacc, mybir
from concourse.bass import ts, ds
from concourse.bass_utils import run_bass_kernel_spmd

F32 = mybir.dt.float32
BF16 = mybir.dt.bfloat16
F8 = mybir.dt.float8e4
I8 = mybir.dt.int8

B, C, H, W = 2, 64, 96, 96
N = H * W                    # 9216
I = 32                       # inter dim
NB = 16                      # gate bottleneck dim
NCORES = 8
CPB = NCORES // B            # cores per batch = 4
QPC = N // CPB               # 2304 query rows per core
KT = 128                     # kv tile
NKV = N // KT                # 72
NPAIR = NKV // 2             # 36 DoubleRow pairs
SLOT = 3                     # kv tiles per QK slot (3-way row tiling)
NSLOT = NKV // SLOT          # 24 slots
NW = NSLOT // 2              # 12 windows (2 slots = 3 PV pairs each)
QCH = 512                    # q chunk (PSUM free dim)
GTS = 48                     # gt2 plane stride (34 used; %16==0 for DoubleRow)
TEMP = 1.5
PR = 0.8
D0 = 16.0 * N                # denominator scale (ones column is 16.0)
A_SCH = 8.0 / (TEMP * math.log(2.0))   # Schraudolph slope for fp8e4 codes
B_SCH = 56.0                           # 7 (bias) * 8

# exp-slot engine assignment: even slots (PSUM buffer A) on ACT, odd slots
# (buffer B) on DVE.  Each window's two exps then always run in parallel on
# different engines, and the pipeline's critical cycle (QK slot -> exp ->
# QK slot two later, bounded by the 2 st buffers) takes the cheaper engine
# on each buffer lane.


def _chunks():
    out = []
    q = 0
    while q < QPC:
        out.append((q, min(QCH, QPC - q)))
        q += QCH
    return out


def _emit(tc, nc, dr, out_d):
    mm = nc.tensor.matmul
    with (
        tc.tile_pool(name="consts", bufs=1) as consts,
        tc.tile_pool(name="work", bufs=2) as work,
        tc.tile_pool(name="epool", bufs=3) as epool,
    ):
        # ---- persistent SBUF tensors -------------------------------------
        xb_sb = consts.tile([C, N], F32)        # residual path
        xbh2_sb = consts.tile([128, N], BF16)   # x stacked twice on partitions
        wbf_sb = consts.tile([128, 352], BF16)  # bf16 weight blob
        thw_sb = wbf_sb[:, 0:128]               # 0.5 * theta_w^T tiled (2, 4)
        phw_sb = wbf_sb[:, 128:256]             # 0.5 * phi_w^T tiled (2, 4)
        gw_sb = wbf_sb[:, 256:288]              # 0.5 * g_w^T tiled (2, 1)
        ww_sb = wbf_sb[:I, 288:352]             # W_w^T
        wf32_sb = consts.tile([C, 82], F32)     # f32 weight blob
        c1w_sb = wf32_sb[:, 0:NB]
        c1b_sb = wf32_sb[:NB, NB : NB + 1]
        c2w_sb = wf32_sb[:NB, 17:81]
        nc2b_sb = wf32_sb[:, 81:82]

        theta4_sb = consts.tile([128, QPC], BF16)   # theta replicated x4
        phi4_sb = consts.tile([128, N], BF16)       # phi replicated x4
        gt2_sb = consts.tile([128, NPAIR, 2, GTS], F8)  # g*16 | ones*16 | 0pad
        gate_sb = consts.tile([C, 1], F32)
        pool4_sb = consts.tile([C, 4], F32)
        pool_sb = consts.tile([C, 1], F32)
        h_sb = consts.tile([NB, 1], F32)
        eg_sb = consts.tile([C, 1], F32)
        nbias_sb = consts.tile([1, 1], F32)     # -1.5 bias for the Square op
        nc.vector.memset(nbias_sb, -1.5)
        c075_sb = consts.tile([C, 1], F32)      # +0.75 of the recip poly
        nc.vector.memset(c075_sb, 0.75)

        nc.sync.dma_start(out=wbf_sb, in_=dr["wbf"])
        for k in range(CPB):
            s0 = k * QPC
            nc.sync.dma_start(out=xbh2_sb[:, s0 : s0 + QPC],
                              in_=dr["xbh2"][:, s0 : s0 + QPC])
        nc.sync.dma_start(out=xb_sb, in_=dr["xb"])
        nc.sync.dma_start(out=wf32_sb, in_=dr["wf32"])

        # ones*16 column + zero pad of the DoubleRow g blob
        nc.vector.memset(gt2_sb[:, :, :, I : I + 1], 16.0)
        nc.vector.memset(gt2_sb[:, :, :, I + 1 : GTS], 0.0)

        # ---- prologue projections (all K=128) ----------------------------
        with tc.tile_pool(name="ppsum", bufs=3, space="PSUM") as pp:
            def proj(dst, w_sb, total, eng_split):
                # two 512-wide matmuls into one 2-bank tile, one wide cast;
                # psum->sbuf casts alternate ACT/DVE to halve the copy tail
                q = 0
                it = 0
                while q < total:
                    pt = pp.tile([128, 2, QCH], F32, tag="pp")
                    n = 0
                    for j in range(2):
                        if q + n < total:
                            w = min(QCH, total - (q + n))
                            mm(out=pt[:, j, :w], lhsT=w_sb,
                               rhs=xbh2_sb[:, ds(q + n, w)],
                               start=True, stop=True)
                            n += w
                    src = pt.rearrange("p a b -> p (a b)")[:, :n]
                    if it % 2 == eng_split:
                        nc.scalar.copy(out=dst[:, ds(q, n)], in_=src)
                    else:
                        nc.vector.tensor_copy(out=dst[:, ds(q, n)], in_=src)
                    q += n
                    it += 1
            proj(theta4_sb, thw_sb, QPC, 0)
            proj(phi4_sb, phw_sb, N, 1)
            done = 0
            while done < NKV:
                nt = min(16, NKV - done)
                pt = pp.tile([128, QCH], F32, tag="pp")
                for k in range(nt):
                    t = done + k
                    mm(out=pt[:, ts(k, I)], lhsT=xbh2_sb[:, ts(t, KT)],
                       rhs=gw_sb, start=True, stop=True)
                nc.vector.tensor_scalar_mul(
                    gt2_sb[:, done // 2 : (done + nt) // 2, :, 0:I],
                    pt[:, : nt * I].rearrange("p (r j i) -> p r j i", j=2, i=I),
                    16.0,
                )
                done += nt
            # gate pooling: bf16 partial reduces (2x DVE rate), after the
            # projection copies so they don't delay the main loop's start
            for k in range(CPB):
                nc.vector.reduce_sum(out=pool4_sb[:, k : k + 1],
                                     in_=xbh2_sb[:C, ts(k, QPC)],
                                     axis=mybir.AxisListType.X)
            nc.vector.reduce_sum(out=pool_sb, in_=pool4_sb,
                                 axis=mybir.AxisListType.X)

        # ---- main loop ---------------------------------------------------
        with (
            tc.tile_pool(name="pst", bufs=2, space="PSUM") as pst,
            tc.tile_pool(name="py", bufs=1, space="PSUM") as py,
            tc.tile_pool(name="pmisc", bufs=1, space="PSUM") as pmisc,
        ):
            def emit_gate():
                # channel gate; emitted after chunk 0's first windows so its
                # matmuls never block the PE queue ahead of the main stream.
                # gate_sb = PR * sigmoid(z) / D0  (the 1/D0 of the softmax
                # denominator normalize folds in here).
                h_ps = pmisc.tile([128, QCH], F32, tag="m")
                mm(out=h_ps[:NB, 0:1], lhsT=c1w_sb, rhs=pool_sb,
                   start=True, stop=True)
                nc.scalar.activation(out=h_sb, in_=h_ps[:NB, 0:1],
                                     func=mybir.ActivationFunctionType.Relu,
                                     bias=c1b_sb, scale=1.0 / float(N))
                z_ps = pmisc.tile([128, QCH], F32, tag="m")
                mm(out=z_ps[:C, 0:1], lhsT=c2w_sb, rhs=h_sb,
                   start=True, stop=True)
                nc.scalar.activation(out=eg_sb, in_=z_ps[:C, 0:1],
                                     func=mybir.ActivationFunctionType.Exp,
                                     bias=nc2b_sb, scale=-1.0)
                nc.vector.tensor_scalar_add(gate_sb, eg_sb, 1.0)
                nc.vector.reciprocal(out=gate_sb, in_=gate_sb)
                nc.vector.tensor_scalar_mul(gate_sb, gate_sb, PR / D0)

            def _epi_head(y_ps, qn):
                ysum = work.tile([I, QCH], BF16, tag="ysum")
                nc.vector.tensor_copy(out=ysum[:, :qn], in_=y_ps[:I, :qn])
                # q = (d/D0 - 1.5)^2, read straight from the PSUM denom row;
                # later gate*(q + 0.75)*o = gate_full*o/d to 3rd order
                q_sb = work.tile([1, QCH], F32, tag="q")
                nc.scalar.activation(out=q_sb[:, :qn],
                                     in_=y_ps[I : I + 1, :qn],
                                     func=mybir.ActivationFunctionType.Square,
                                     bias=nbias_sb, scale=1.0 / D0)
                bc = work.tile([C, QCH], F32, tag="bc")
                nc.gpsimd.partition_broadcast(bc[:, :qn], q_sb[:, :qn])
                return ysum, bc

            def _epi_tail(qs, qn, ysum, bc):
                o_ps = pmisc.tile([128, QCH], F32, tag="m")
                mm(out=o_ps[:C, :qn], lhsT=ww_sb, rhs=ysum[:, :qn],
                   start=True, stop=True)
                t1 = work.tile([C, QCH], F32, tag="t1")
                nc.vector.scalar_tensor_tensor(
                    out=t1[:, :qn], in0=bc[:, :qn], scalar=0.75,
                    in1=o_ps[:C, :qn],
                    op0=mybir.AluOpType.add, op1=mybir.AluOpType.mult)
                out_sb = work.tile([C, QCH], F32, tag="out")
                nc.vector.scalar_tensor_tensor(
                    out=out_sb[:, :qn], in0=t1[:, :qn], scalar=gate_sb,
                    in1=xb_sb[:, ds(qs, qn)],
                    op0=mybir.AluOpType.mult, op1=mybir.AluOpType.add)
                nc.sync.dma_start(out=out_d[:, ds(qs, qn)],
                                  in_=out_sb[:, :qn])

            chunks = _chunks()
            pend_pv = None     # last window's PV matmuls (deferred 1 window)
            pend_head = None   # previous chunk's epilogue head
            pend_tail = None   # previous chunk's epilogue tail
            for ci, (qs, qn) in enumerate(chunks):
                y_ps = py.tile([I + 2, QCH], F32, tag="y")
                # previous chunk's last PV + epilogue head go FIRST so the
                # head's copies land at the front of the DVE/ACT queues and
                # release the y bank before this chunk's PV pair 0 needs it.
                if pend_head is not None:
                    pend_head()
                    pend_head = None
                for w in range(NW):
                    e_t = epool.tile([128, 2 * SLOT, QCH], F8, tag="e")
                    for s2 in range(2):
                        slot = 2 * w + s2
                        st = pst.tile([128, SLOT, QCH], F32, tag="st")
                        for j in range(SLOT):
                            t = SLOT * slot + j
                            mm(out=st[:, j, :qn],
                               lhsT=phi4_sb[32 * j : 32 * j + 32, ts(t, KT)],
                               rhs=theta4_sb[32 * j : 32 * j + 32, ds(qs, qn)],
                               start=True, stop=True)
                        if qn == QCH:
                            # flat 2D APs (contiguous planes) stream best
                            eo = e_t[:, SLOT * s2 : SLOT * s2 + SLOT, :]
                            eo = eo.rearrange("p a b -> p (a b)")
                            si = st.rearrange("p a b -> p (a b)")
                        else:
                            eo = e_t[:, SLOT * s2 : SLOT * s2 + SLOT, :qn]
                            si = st[:, :, :qn]
                        if slot % 2 == 0:
                            nc.scalar.activation(
                                out=eo, in_=si,
                                func=mybir.ActivationFunctionType.Exp,
                                scale=1.0 / TEMP)
                        else:
                            nc.vector.tensor_scalar(
                                out=eo.bitcast(I8), in0=si,
                                scalar1=A_SCH, scalar2=B_SCH,
                                op0=mybir.AluOpType.mult,
                                op1=mybir.AluOpType.add)
                    if pend_pv is not None:
                        pend_pv()
                    if pend_tail is not None and w == 3:
                        pend_tail()
                        pend_tail = None

                    def _pv(e_t=e_t, w=w, y_ps=y_ps, qn=qn):
                        for p in range(3):
                            pr = 3 * w + p
                            mm(out=y_ps[: I + 2, :qn],
                               lhsT=gt2_sb[:, pr, :, : I + 2],
                               rhs=e_t[:, 2 * p : 2 * p + 2, :qn],
                               start=(pr == 0), stop=(pr == NPAIR - 1),
                               perf_mode=mybir.MatmulPerfMode.DoubleRow)
                    pend_pv = _pv
                if ci == 0:
                    emit_gate()

                if ci + 1 < len(chunks):
                    # defer: last window's PV + head into next chunk's stream
                    def _head(y_ps=y_ps, qs=qs, qn=qn, pv=pend_pv):
                        pv()
                        ysum, bc = _epi_head(y_ps, qn)

                        def _tail(qs=qs, qn=qn, ysum=ysum, bc=bc):
                            _epi_tail(qs, qn, ysum, bc)

                        nonlocal pend_tail
                        pend_tail = _tail
                    pend_pv = None
                    pend_head = _head
                else:
                    pend_pv()
                    pend_pv = None
                    ysum, bc = _epi_head(y_ps, qn)
                    _epi_tail(qs, qn, ysum, bc)
            if pend_head is not None:
                pend_head()
            if pend_tail is not None:
                pend_tail()


def build():
    nc = bacc.Bacc("TRN2", target_bir_lowering=False, debug=False)
    names = {
        "xb": ([C, N], F32), "xbh2": ([128, N], BF16),
        "wbf": ([128, 352], BF16), "wf32": ([C, 82], F32),
    }
    dr = {k: nc.dram_tensor(k, shp, dt, kind="ExternalInput").ap()
          for k, (shp, dt) in names.items()}
    out_d = nc.dram_tensor("out", [C, QPC], F32, kind="ExternalOutput").ap()
    with tile.TileContext(nc) as tc:
        _emit(tc, nc, dr, out_d)
    nc.compile()
    return nc


_NC = None


def _get_nc():
    global _NC
    if _NC is None:
        _NC = build()
    return _NC


def make_in_maps(inputs):
    bf = ml_dtypes.bfloat16
    xf = np.ascontiguousarray(np.asarray(inputs["x"], np.float32).reshape(B, C, N))
    thwT = np.asarray(inputs["theta_w"], np.float32).T        # [C, I]
    phwT = np.asarray(inputs["phi_w"], np.float32).T
    gwT = np.asarray(inputs["g_w"], np.float32).T
    wbf = np.zeros((128, 352), np.float32)
    wbf[:, 0:128] = np.tile(thwT, (2, 4)) * 0.5
    wbf[:, 128:256] = np.tile(phwT, (2, 4)) * 0.5
    wbf[:, 256:288] = np.tile(gwT, (2, 1)) * 0.5
    wbf[:I, 288:352] = np.asarray(inputs["W_w"], np.float32).T
    wf32 = np.zeros((C, 82), np.float32)
    wf32[:, 0:NB] = np.asarray(inputs["cg1_w"], np.float32).T
    wf32[:NB, NB] = np.asarray(inputs["cg1_b"], np.float32)
    wf32[:NB, 17:81] = np.asarray(inputs["cg2_w"], np.float32).T
    wf32[:, 81] = -np.asarray(inputs["cg2_b"], np.float32)
    shared = {"wbf": wbf.astype(bf), "wf32": wf32}
    in_maps = []
    for core in range(NCORES):
        b, q0 = core // CPB, (core % CPB) * QPC
        m = dict(shared)
        xr = np.ascontiguousarray(np.roll(xf[b], -q0, axis=1))
        m["xb"] = xr
        m["xbh2"] = np.ascontiguousarray(np.tile(xr, (2, 1))).astype(bf)
        in_maps.append(m)
    return in_maps


def gather(results):
    y = np.empty((B, C, N), np.float32)
    for core in range(NCORES):
        b, q0 = core // CPB, (core % CPB) * QPC
        y[b][:, q0 : q0 + QPC] = results[core]["out"]
    return y.reshape(B, C, H, W)


def run(inputs, trace=False, **kw):
    res = run_bass_kernel_spmd(_get_nc(), make_in_maps(inputs),
                               core_ids=list(range(NCORES)), trace=trace, **kw)
    return gather(res.results), res


def kernel(**inputs):
    out, _ = run(inputs)
    return out
